# revision 1
# baseline (speedup 1.0000x reference)
"""BnFireFc fused kernel for 8 Trainium2 NeuronCores.

Math: training-mode BN over N, then heaviside spike, then FC:
    mean/var over axis 0 of x [N, C]
    y = (x - mean) * rsqrt(var + eps) * w_bn + b_bn
    spike = (y > 0)
    out = spike @ W_fc.T + b_fc

Since w_bn > 0 and rsqrt > 0, spike[n,c] == (x[n,c] > t[c]) with
    t[c] = mean[c] - b_bn[c] * sqrt(var[c] + eps) / w_bn[c]
so BN+Fire collapses to one per-channel threshold compare.

Distribution: data-parallel over N across 8 cores. Each core receives its
x-shard pre-transposed on host to [C, n_loc] so channels sit on SBUF
partitions (4 blocks of 128): pass 1 accumulates per-channel sum(x) and
sum(x^2) along the free dim (DVE / ACT accum_out), a 4 KB AllReduce merges
the partial stats, then pass 2 re-streams x^T, compares against t[c]
(per-partition tensor_scalar, exact fp32) into an fp16 0/1 spike, and
accumulates out[n,:] = sum_c spike^T[c,n] * W_fc.T[c,:] on the PE in fp16
(spike values are exact in fp16; only W rounds). b_fc is added on host.
"""

import os
import numpy as np

import concourse.bass as bass
import concourse.tile as tile
from concourse import bacc, mybir
from concourse.bass_utils import run_bass_kernel_spmd

dt = mybir.dt
Alu = mybir.AluOpType
Act = mybir.ActivationFunctionType

NCORES = 8
N_FULL = 131072
C = 512          # cin
O = 512          # cout
NB = C // 128    # channel blocks on partitions
EPS_BN = 1e-5

N_LOC = N_FULL // NCORES   # 16384 rows per core
F1 = 2048                  # pass-1 tile free size (n elems)
F2 = 2048                  # pass-2 tile free size


def build_program(n_loc=N_LOC, ncores=NCORES):
    nj1 = n_loc // F1              # pass-1 n-chunks per c-block
    nj2 = n_loc // F2              # pass-2 n-chunks
    nsub = F2 // 128               # 128-row subtiles per chunk

    nc = bacc.Bacc("TRN2", target_bir_lowering=False, debug=False,
                   num_devices=ncores)

    xt_d = nc.dram_tensor("xt", [C, n_loc], dt.float32, kind="ExternalInput").ap()
    wt_d = nc.dram_tensor("wt", [C, O], dt.float16, kind="ExternalInput").ap()
    wbn_d = nc.dram_tensor("wbn", [128, NB], dt.float32, kind="ExternalInput").ap()
    bbn_d = nc.dram_tensor("bbn", [128, NB], dt.float32, kind="ExternalInput").ap()
    out_d = nc.dram_tensor("out", [n_loc, O], dt.float32, kind="ExternalOutput").ap()

    with tile.TileContext(nc) as tc:
        with (
            tc.tile_pool(name="ld1", bufs=4) as ld1,
            tc.tile_pool(name="ld2", bufs=4) as ld2,
            tc.tile_pool(name="spk", bufs=6) as spk,
            tc.tile_pool(name="wts", bufs=1) as wts,
            tc.tile_pool(name="stat", bufs=1) as stat,
            tc.tile_pool(name="scr", bufs=1) as scr,
            tc.tile_pool(name="outp", bufs=4) as outp,
            tc.tile_pool(name="ps", bufs=6, space="PSUM") as ps,
            tc.tile_pool(name="dram", bufs=1, space="DRAM") as dram,
        ):
            # ---- replicated weights ----
            wt_sb = wts.tile([128, NB, O], dt.float16)      # W_fc.T, c-blocked
            for b in range(NB):
                nc.sync.dma_start(wt_sb[:, b, :], wt_d[b * 128:(b + 1) * 128, :])
            wbn = wts.tile([128, NB], dt.float32)
            bbn = wts.tile([128, NB], dt.float32)
            nc.sync.dma_start(wbn[:], wbn_d[:])
            nc.sync.dma_start(bbn[:], bbn_d[:])

            # ---- pass 1: per-channel sum(x), sum(x^2) partials ----
            s1c = stat.tile([128, NB, nj1], dt.float32)
            s2c = stat.tile([128, NB, nj1], dt.float32)
            scr_d = scr.tile([128, F1], dt.bfloat16)   # DVE dummy main-out
            scr_a = scr.tile([128, F1], dt.bfloat16)   # ACT dummy main-out
            for b in range(NB):
                for j in range(nj1):
                    xt = ld1.tile([128, F1], dt.float32)
                    nc.sync.dma_start(
                        xt[:], xt_d[b * 128:(b + 1) * 128, j * F1:(j + 1) * F1])
                    nc.vector.tensor_scalar(
                        scr_d[:], xt[:], 1.0, None, Alu.mult, Alu.add,
                        accum_out=s1c[:, b, j:j + 1])
                    nc.scalar.activation(
                        scr_a[:], xt[:], Act.Square,
                        accum_out=s2c[:, b, j:j + 1])

            # reduce partials -> [128, NB], pack into [128, 2*NB]
            st_loc = stat.tile([128, 2 * NB], dt.float32)
            nc.vector.tensor_reduce(
                st_loc[:, 0:NB], s1c[:], axis=mybir.AxisListType.X, op=Alu.add)
            nc.vector.tensor_reduce(
                st_loc[:, NB:2 * NB], s2c[:], axis=mybir.AxisListType.X, op=Alu.add)

            # ---- AllReduce the 4 KB stats across cores ----
            st_in = dram.tile([128, 2 * NB], dt.float32)
            st_out = dram.tile([128, 2 * NB], dt.float32)
            nc.sync.dma_start(st_in[:], st_loc[:])
            nc.gpsimd.collective_compute(
                "AllReduce", Alu.add,
                replica_groups=[list(range(ncores))],
                ins=[st_in.opt()], outs=[st_out.opt()],
            )
            st_g = stat.tile([128, 2 * NB], dt.float32)
            nc.sync.dma_start(st_g[:], st_out[:])

            # ---- threshold t = mean - b_bn * sqrt(var+eps) / w_bn ----
            inv_n = 1.0 / (n_loc * ncores)
            mean = stat.tile([128, NB], dt.float32)
            varpe = stat.tile([128, NB], dt.float32)
            tmp = stat.tile([128, NB], dt.float32)
            tmp2 = stat.tile([128, NB], dt.float32)
            sd = stat.tile([128, NB], dt.float32)
            thr = stat.tile([128, NB], dt.float32)
            nc.vector.tensor_scalar_mul(mean[:], st_g[:, 0:NB], inv_n)
            nc.vector.tensor_scalar_mul(varpe[:], st_g[:, NB:2 * NB], inv_n)
            nc.vector.tensor_tensor(tmp[:], mean[:], mean[:], Alu.mult)
            nc.vector.tensor_tensor(varpe[:], varpe[:], tmp[:], Alu.subtract)
            nc.vector.tensor_scalar_add(varpe[:], varpe[:], EPS_BN)
            # seed sqrt on ACT (low-precision table), then 2 Newton steps
            # with the accurate DVE reciprocal: s <- 0.5*(s + v/s)
            nc.scalar.activation(sd[:], varpe[:], Act.Sqrt)
            for _ in range(2):
                nc.vector.reciprocal(tmp[:], sd[:])
                nc.vector.tensor_tensor(tmp[:], varpe[:], tmp[:], Alu.mult)
                nc.vector.tensor_tensor(tmp[:], sd[:], tmp[:], Alu.add)
                nc.vector.tensor_scalar_mul(sd[:], tmp[:], 0.5)
            nc.vector.reciprocal(tmp[:], wbn[:])
            nc.vector.tensor_tensor(tmp2[:], bbn[:], sd[:], Alu.mult)
            nc.vector.tensor_tensor(tmp[:], tmp2[:], tmp[:], Alu.mult)
            nc.vector.tensor_tensor(thr[:], mean[:], tmp[:], Alu.subtract)

            # ---- pass 2: spike + matmul ----
            for j in range(nj2):
                sps = []
                for b in range(NB):
                    xt = ld2.tile([128, F2], dt.float32)
                    nc.sync.dma_start(
                        xt[:], xt_d[b * 128:(b + 1) * 128, j * F2:(j + 1) * F2])
                    sp = spk.tile([128, F2], dt.float16)
                    nc.vector.tensor_scalar(
                        sp[:], xt[:], thr[:, b:b + 1], None, Alu.is_gt)
                    sps.append(sp)
                for s in range(nsub):
                    acc = ps.tile([128, O], dt.float32)
                    for b in range(NB):
                        nc.tensor.matmul(
                            acc[:], sps[b][:, s * 128:(s + 1) * 128],
                            wt_sb[:, b, :], start=(b == 0), stop=(b == NB - 1))
                    o_sb = outp.tile([128, O], dt.float32)
                    nc.scalar.copy(o_sb[:], acc[:])
                    row = j * F2 + s * 128
                    nc.sync.dma_start(out_d[row:row + 128, :], o_sb[:])

    nc.compile()
    return nc


_PROGRAM_CACHE = {}


def _get_program(n_loc=N_LOC, ncores=NCORES):
    key = (n_loc, ncores)
    if key not in _PROGRAM_CACHE:
        _PROGRAM_CACHE[key] = build_program(n_loc, ncores)
    return _PROGRAM_CACHE[key]


def _prep_inputs(x, weight_bn, bias_bn, weight_fc, ncores=NCORES):
    n = x.shape[0]
    n_loc = n // ncores
    wt = np.ascontiguousarray(weight_fc.T).astype(np.float16)
    wbn = np.ascontiguousarray(weight_bn.reshape(NB, 128).T)
    bbn = np.ascontiguousarray(bias_bn.reshape(NB, 128).T)
    xt_full = np.ascontiguousarray(x.T)  # [C, N]
    in_maps = []
    for i in range(ncores):
        xt_i = np.ascontiguousarray(xt_full[:, i * n_loc:(i + 1) * n_loc])
        in_maps.append({"xt": xt_i, "wt": wt, "wbn": wbn, "bbn": bbn})
    return in_maps


def kernel(x, weight_bn, bias_bn, weight_fc, bias_fc, _trace=False):
    x = np.asarray(x, dtype=np.float32)
    weight_bn = np.asarray(weight_bn, dtype=np.float32)
    bias_bn = np.asarray(bias_bn, dtype=np.float32)
    weight_fc = np.asarray(weight_fc, dtype=np.float32)
    bias_fc = np.asarray(bias_fc, dtype=np.float32)

    n = x.shape[0]
    ncores = NCORES
    n_loc = n // ncores
    nc = _get_program(n_loc, ncores)
    in_maps = _prep_inputs(x, weight_bn, bias_bn, weight_fc, ncores)
    res = run_bass_kernel_spmd(
        nc, in_maps, list(range(ncores)), trace=_trace)
    out = np.concatenate([res.results[i]["out"] for i in range(ncores)], axis=0)
    out += bias_fc[None, :]
    if _trace:
        kernel._last_results = res
    return out


if __name__ == "__main__":
    # quick self-check at reduced size (still 8 cores)
    rng = np.random.default_rng(0)
    n = int(os.environ.get("KERNEL_SELFTEST_N", 16384))
    x = rng.standard_normal((n, C)).astype(np.float32)
    wbn = rng.uniform(0.5, 1.5, C).astype(np.float32)
    bbn = (rng.standard_normal(C) * 0.1).astype(np.float32)
    wfc = rng.uniform(-1 / np.sqrt(C), 1 / np.sqrt(C), (O, C)).astype(np.float32)
    bfc = rng.uniform(-1 / np.sqrt(C), 1 / np.sqrt(C), O).astype(np.float32)

    mean = x.mean(axis=0)
    var = ((x - mean) ** 2).mean(axis=0)
    y = (x - mean) / np.sqrt(var + EPS_BN) * wbn + bbn
    spike = (y > 0).astype(np.float32)
    expected = spike @ wfc.T + bfc

    actual = kernel(x, wbn, bbn, wfc, bfc)
    err = np.abs(actual - expected).max() / np.abs(expected).max()
    print(f"selftest n={n}: rel_err={err:.3e}")


# revision 2
# speedup vs baseline: 1.2314x; 1.2314x over previous
"""BnFireFc fused kernel for 8 Trainium2 NeuronCores.

Math: training-mode BN over N, then heaviside spike, then FC:
    mean/var over axis 0 of x [N, C]
    y = (x - mean) * rsqrt(var + eps) * w_bn + b_bn
    spike = (y > 0)
    out = spike @ W_fc.T + b_fc

Since w_bn > 0 and rsqrt > 0, spike[n,c] == (x[n,c] > t[c]) with
    t[c] = mean[c] - b_bn[c] * sqrt(var[c] + eps) / w_bn[c]
so BN+Fire collapses to one per-channel threshold compare.

Distribution: data-parallel over N across 8 cores. Each core receives its
x-shard pre-transposed on host to [C, n_loc] so channels sit on SBUF
partitions (4 blocks of 128): pass 1 accumulates per-channel sum(x) and
sum(x^2) along the free dim (DVE / ACT accum_out), a 4 KB AllReduce merges
the partial stats, then pass 2 streams x^T again, compares against t[c]
(per-partition tensor_scalar, exact fp32) into an fp16 0/1 spike, and
accumulates out[n,:] = sum_c spike^T[c,n] * W_fc.T[c,:] on the PE in fp16
(spike values are exact in fp16; only W rounds). b_fc is added on host.

HBM-traffic optimizations: the last CACHE_J chunks of pass 1 stay resident
in SBUF and are not re-read in pass 2; the output is written as fp16 and
upcast on host; deep pass-2 load buffering lets the DMA engines prefetch
through the AllReduce sync bubble.
"""

import os
import numpy as np

import concourse.bass as bass
import concourse.tile as tile
from concourse import bacc, mybir
from concourse.bass_utils import run_bass_kernel_spmd

dt = mybir.dt
Alu = mybir.AluOpType
Act = mybir.ActivationFunctionType

NCORES = 8
N_FULL = 131072
C = 512          # cin
O = 512          # cout
NB = C // 128    # channel blocks on partitions
EPS_BN = 1e-5

N_LOC = N_FULL // NCORES   # 16384 rows per core
F = 2048                   # tile free size (n elems per chunk column)
CACHE_J = 2                # trailing j-chunks kept in SBUF between passes


def build_program(n_loc=N_LOC, ncores=NCORES):
    nj = n_loc // F               # j-chunks
    nsub = F // 128               # 128-row subtiles per chunk
    cache_j = min(CACHE_J, nj)
    cached = set(range(nj - cache_j, nj))

    nc = bacc.Bacc("TRN2", target_bir_lowering=False, debug=False,
                   num_devices=ncores)

    xt_d = nc.dram_tensor("xt", [C, n_loc], dt.float32, kind="ExternalInput").ap()
    wt_d = nc.dram_tensor("wt", [C, O], dt.float16, kind="ExternalInput").ap()
    wbn_d = nc.dram_tensor("wbn", [128, NB], dt.float32, kind="ExternalInput").ap()
    bbn_d = nc.dram_tensor("bbn", [128, NB], dt.float32, kind="ExternalInput").ap()
    out_d = nc.dram_tensor("out", [n_loc, O], dt.float16, kind="ExternalOutput").ap()

    with tile.TileContext(nc) as tc:
        with (
            tc.tile_pool(name="ld1", bufs=3) as ld1,
            tc.tile_pool(name="ld2", bufs=8) as ld2,
            tc.tile_pool(name="xcache", bufs=cache_j * NB) as xcache,
            tc.tile_pool(name="spk", bufs=6) as spk,
            tc.tile_pool(name="wts", bufs=1) as wts,
            tc.tile_pool(name="stat", bufs=1) as stat,
            tc.tile_pool(name="outp", bufs=4) as outp,
            tc.tile_pool(name="ps", bufs=6, space="PSUM") as ps,
            tc.tile_pool(name="dram", bufs=1, space="DRAM") as dram,
        ):
            # ---- replicated weights ----
            wt_sb = wts.tile([128, NB, O], dt.float16)      # W_fc.T, c-blocked
            for b in range(NB):
                nc.sync.dma_start(wt_sb[:, b, :], wt_d[b * 128:(b + 1) * 128, :])
            wbn = wts.tile([128, NB], dt.float32)
            bbn = wts.tile([128, NB], dt.float32)
            nc.sync.dma_start(wbn[:], wbn_d[:])
            nc.sync.dma_start(bbn[:], bbn_d[:])

            # ---- pass 1: per-channel sum(x), sum(x^2) partials ----
            s1c = stat.tile([128, NB, nj], dt.float32)
            s2c = stat.tile([128, NB, nj], dt.float32)
            scr_d = scr_a = None
            cache_tiles = {}
            for j in range(nj):
                for b in range(NB):
                    pool = xcache if j in cached else ld1
                    xt = pool.tile([128, F], dt.float32)
                    if j in cached:
                        cache_tiles[(j, b)] = xt
                    nc.sync.dma_start(
                        xt[:], xt_d[b * 128:(b + 1) * 128, j * F:(j + 1) * F])
                    if scr_d is None:
                        scr_d = stat.tile([128, 1], dt.bfloat16)
                        scr_a = stat.tile([128, 1], dt.bfloat16)
                    nc.vector.tensor_scalar(
                        scr_d.broadcast_to((128, F)), xt[:], 1.0, None,
                        Alu.mult, Alu.add, accum_out=s1c[:, b, j:j + 1])
                    nc.scalar.activation(
                        scr_a.broadcast_to((128, F)), xt[:], Act.Square,
                        accum_out=s2c[:, b, j:j + 1])

            # reduce partials -> [128, NB], pack into [128, 2*NB]
            st_loc = stat.tile([128, 2 * NB], dt.float32)
            nc.vector.tensor_reduce(
                st_loc[:, 0:NB], s1c[:], axis=mybir.AxisListType.X, op=Alu.add)
            nc.vector.tensor_reduce(
                st_loc[:, NB:2 * NB], s2c[:], axis=mybir.AxisListType.X, op=Alu.add)

            # ---- AllReduce the 4 KB stats across cores ----
            st_in = dram.tile([128, 2 * NB], dt.float32)
            st_out = dram.tile([128, 2 * NB], dt.float32)
            nc.sync.dma_start(st_in[:], st_loc[:])
            nc.gpsimd.collective_compute(
                "AllReduce", Alu.add,
                replica_groups=[list(range(ncores))],
                ins=[st_in.opt()], outs=[st_out.opt()],
            )
            st_g = stat.tile([128, 2 * NB], dt.float32)
            nc.sync.dma_start(st_g[:], st_out[:])

            # ---- threshold t = mean - b_bn * sqrt(var+eps) / w_bn ----
            inv_n = 1.0 / (n_loc * ncores)
            mean = stat.tile([128, NB], dt.float32)
            varpe = stat.tile([128, NB], dt.float32)
            tmp = stat.tile([128, NB], dt.float32)
            tmp2 = stat.tile([128, NB], dt.float32)
            sd = stat.tile([128, NB], dt.float32)
            thr = stat.tile([128, NB], dt.float32)
            nc.vector.tensor_scalar_mul(mean[:], st_g[:, 0:NB], inv_n)
            nc.vector.tensor_scalar_mul(varpe[:], st_g[:, NB:2 * NB], inv_n)
            nc.vector.tensor_tensor(tmp[:], mean[:], mean[:], Alu.mult)
            nc.vector.tensor_tensor(varpe[:], varpe[:], tmp[:], Alu.subtract)
            nc.vector.tensor_scalar_add(varpe[:], varpe[:], EPS_BN)
            # seed sqrt on ACT (low-precision table), then 2 Newton steps
            # with the accurate DVE reciprocal: s <- 0.5*(s + v/s)
            nc.scalar.activation(sd[:], varpe[:], Act.Sqrt)
            for _ in range(2):
                nc.vector.reciprocal(tmp[:], sd[:])
                nc.vector.tensor_tensor(tmp[:], varpe[:], tmp[:], Alu.mult)
                nc.vector.tensor_tensor(tmp[:], sd[:], tmp[:], Alu.add)
                nc.vector.tensor_scalar_mul(sd[:], tmp[:], 0.5)
            nc.vector.reciprocal(tmp[:], wbn[:])
            nc.vector.tensor_tensor(tmp2[:], bbn[:], sd[:], Alu.mult)
            nc.vector.tensor_tensor(tmp[:], tmp2[:], tmp[:], Alu.mult)
            nc.vector.tensor_tensor(thr[:], mean[:], tmp[:], Alu.subtract)

            # ---- pass 2: spike + matmul ----
            for j in range(nj):
                sps = []
                for b in range(NB):
                    if j in cached:
                        xt = cache_tiles[(j, b)]
                    else:
                        xt = ld2.tile([128, F], dt.float32)
                        nc.sync.dma_start(
                            xt[:], xt_d[b * 128:(b + 1) * 128, j * F:(j + 1) * F])
                    sp = spk.tile([128, F], dt.float16)
                    nc.vector.tensor_scalar(
                        sp[:], xt[:], thr[:, b:b + 1], None, Alu.is_gt)
                    sps.append(sp)
                for s in range(nsub):
                    acc = ps.tile([128, O], dt.float32)
                    for b in range(NB):
                        nc.tensor.matmul(
                            acc[:], sps[b][:, s * 128:(s + 1) * 128],
                            wt_sb[:, b, :], start=(b == 0), stop=(b == NB - 1))
                    o_sb = outp.tile([128, O], dt.float16)
                    nc.scalar.copy(o_sb[:], acc[:])
                    row = j * F + s * 128
                    nc.sync.dma_start(out_d[row:row + 128, :], o_sb[:])

    nc.compile()
    return nc


_PROGRAM_CACHE = {}


def _get_program(n_loc=N_LOC, ncores=NCORES):
    key = (n_loc, ncores)
    if key not in _PROGRAM_CACHE:
        _PROGRAM_CACHE[key] = build_program(n_loc, ncores)
    return _PROGRAM_CACHE[key]


def _prep_inputs(x, weight_bn, bias_bn, weight_fc, ncores=NCORES):
    n = x.shape[0]
    n_loc = n // ncores
    wt = np.ascontiguousarray(weight_fc.T).astype(np.float16)
    wbn = np.ascontiguousarray(weight_bn.reshape(NB, 128).T)
    bbn = np.ascontiguousarray(bias_bn.reshape(NB, 128).T)
    xt_full = np.ascontiguousarray(x.T)  # [C, N]
    in_maps = []
    for i in range(ncores):
        xt_i = np.ascontiguousarray(xt_full[:, i * n_loc:(i + 1) * n_loc])
        in_maps.append({"xt": xt_i, "wt": wt, "wbn": wbn, "bbn": bbn})
    return in_maps


def kernel(x, weight_bn, bias_bn, weight_fc, bias_fc, _trace=False):
    x = np.asarray(x, dtype=np.float32)
    weight_bn = np.asarray(weight_bn, dtype=np.float32)
    bias_bn = np.asarray(bias_bn, dtype=np.float32)
    weight_fc = np.asarray(weight_fc, dtype=np.float32)
    bias_fc = np.asarray(bias_fc, dtype=np.float32)

    n = x.shape[0]
    ncores = NCORES
    n_loc = n // ncores
    nc = _get_program(n_loc, ncores)
    in_maps = _prep_inputs(x, weight_bn, bias_bn, weight_fc, ncores)
    res = run_bass_kernel_spmd(
        nc, in_maps, list(range(ncores)), trace=_trace)
    out = np.concatenate(
        [res.results[i]["out"] for i in range(ncores)], axis=0
    ).astype(np.float32)
    out += bias_fc[None, :]
    if _trace:
        kernel._last_results = res
    return out


if __name__ == "__main__":
    # quick self-check at reduced size (still 8 cores)
    rng = np.random.default_rng(0)
    n = int(os.environ.get("KERNEL_SELFTEST_N", 16384))
    x = rng.standard_normal((n, C)).astype(np.float32)
    wbn = rng.uniform(0.5, 1.5, C).astype(np.float32)
    bbn = (rng.standard_normal(C) * 0.1).astype(np.float32)
    wfc = rng.uniform(-1 / np.sqrt(C), 1 / np.sqrt(C), (O, C)).astype(np.float32)
    bfc = rng.uniform(-1 / np.sqrt(C), 1 / np.sqrt(C), O).astype(np.float32)

    mean = x.mean(axis=0)
    var = ((x - mean) ** 2).mean(axis=0)
    y = (x - mean) / np.sqrt(var + EPS_BN) * wbn + bbn
    spike = (y > 0).astype(np.float32)
    expected = spike @ wfc.T + bfc

    actual = kernel(x, wbn, bbn, wfc, bfc)
    err = np.abs(actual - expected).max() / np.abs(expected).max()
    print(f"selftest n={n}: rel_err={err:.3e}")


# revision 8
# speedup vs baseline: 1.3314x; 1.0812x over previous
"""BnFireFc fused kernel for 8 Trainium2 NeuronCores.

Math: training-mode BN over N, then heaviside spike, then FC:
    mean/var over axis 0 of x [N, C]
    y = (x - mean) * rsqrt(var + eps) * w_bn + b_bn
    spike = (y > 0)
    out = spike @ W_fc.T + b_fc

Since w_bn > 0 and rsqrt > 0, spike[n,c] == (x[n,c] > t[c]) with
    t[c] = mean[c] - b_bn[c] * sqrt(var[c] + eps) / w_bn[c]
so BN+Fire collapses to one per-channel threshold compare.

Distribution: data-parallel over N across 8 cores. Each core receives its
x-shard pre-transposed on host to [C, n_loc] so channels sit on SBUF
partitions (4 blocks of 128): pass 1 accumulates per-channel sum(x) and
sum(x^2) along the free dim (DVE / ACT accum_out), a 4 KB AllReduce merges
the partial stats, then pass 2 streams x^T again, compares against t[c]
(per-partition tensor_scalar, exact fp32) into an fp16 0/1 spike, and
accumulates out[n,:] = sum_c spike^T[c,n] * W_fc.T[c,:] on the PE in fp16
(spike values are exact in fp16; only W rounds). b_fc is added on host.

HBM-traffic optimizations: the last CACHE_J chunks of pass 1 stay resident
in SBUF and are not re-read in pass 2; the output is written as fp16 and
upcast on host; deep pass-2 load buffering lets the DMA engines prefetch
through the AllReduce sync bubble.
"""

import os
import numpy as np

import concourse.bass as bass
import concourse.tile as tile
from concourse import bacc, mybir
from concourse.bass_utils import run_bass_kernel_spmd

dt = mybir.dt
Alu = mybir.AluOpType
Act = mybir.ActivationFunctionType

NCORES = 8
N_FULL = 131072
C = 512          # cin
O = 512          # cout
NB = C // 128    # channel blocks on partitions
EPS_BN = 1e-5

N_LOC = N_FULL // NCORES   # 16384 rows per core
F1 = 4096                  # pass-1 tile free size (n elems)
F = 2048                   # pass-2 tile free size (n elems per chunk column)


def build_program(n_loc=N_LOC, ncores=NCORES):
    f1 = min(F1, n_loc)
    f = min(F, n_loc)
    nj1 = n_loc // f1             # pass-1 j-chunks
    nj = n_loc // f               # pass-2 j-chunks
    nsub = f // 128               # 128-row subtiles per chunk

    nc = bacc.Bacc("TRN2", target_bir_lowering=False, debug=False,
                   num_devices=ncores)

    xt_d = nc.dram_tensor("xt", [C, n_loc], dt.float32, kind="ExternalInput").ap()
    wt_d = nc.dram_tensor("wt", [C, O], dt.float16, kind="ExternalInput").ap()
    wbn_d = nc.dram_tensor("wbn", [128, NB], dt.float32, kind="ExternalInput").ap()
    bbn_d = nc.dram_tensor("bbn", [128, NB], dt.float32, kind="ExternalInput").ap()
    out_d = nc.dram_tensor("out", [n_loc, O], dt.float16, kind="ExternalOutput").ap()

    with tile.TileContext(nc) as tc:
        with (
            tc.tile_pool(name="ld1", bufs=2) as ld1,
            tc.tile_pool(name="ld2", bufs=15) as ld2,
            tc.tile_pool(name="spk", bufs=8) as spk,
            tc.tile_pool(name="wts", bufs=1) as wts,
            tc.tile_pool(name="stat", bufs=1) as stat,
            tc.tile_pool(name="outp", bufs=3) as outp,
            tc.tile_pool(name="ps", bufs=6, space="PSUM") as ps,
            tc.tile_pool(name="dram", bufs=1, space="DRAM") as dram,
        ):
            # ---- replicated weights ----
            wt_sb = wts.tile([128, NB, O], dt.float16)      # W_fc.T, c-blocked
            for b in range(NB):
                nc.sync.dma_start(wt_sb[:, b, :], wt_d[b * 128:(b + 1) * 128, :])
            wbn = wts.tile([128, NB], dt.float32)
            bbn = wts.tile([128, NB], dt.float32)
            nc.sync.dma_start(wbn[:], wbn_d[:])
            nc.sync.dma_start(bbn[:], bbn_d[:])

            # ---- pass 1: per-channel sum(x), sum(x^2) partials ----
            s1c = stat.tile([128, NB, nj1], dt.float32)
            s2c = stat.tile([128, NB, nj1], dt.float32)
            scr_d = stat.tile([128, 1], dt.bfloat16)
            scr_a = stat.tile([128, 1], dt.bfloat16)
            for j in range(nj1):
                for b in range(NB):
                    xt = ld1.tile([128, f1], dt.float32)
                    nc.sync.dma_start(
                        xt[:], xt_d[b * 128:(b + 1) * 128, j * f1:(j + 1) * f1])
                    nc.vector.tensor_scalar(
                        scr_d.broadcast_to((128, f1)), xt[:], 1.0, None,
                        Alu.mult, Alu.add, accum_out=s1c[:, b, j:j + 1])
                    nc.scalar.activation(
                        scr_a.broadcast_to((128, f1)), xt[:], Act.Square,
                        accum_out=s2c[:, b, j:j + 1])

            # reduce partials -> [128, NB], pack into [128, 2*NB]
            st_loc = stat.tile([128, 2 * NB], dt.float32)
            nc.vector.tensor_reduce(
                st_loc[:, 0:NB], s1c[:], axis=mybir.AxisListType.X, op=Alu.add)
            nc.vector.tensor_reduce(
                st_loc[:, NB:2 * NB], s2c[:], axis=mybir.AxisListType.X, op=Alu.add)

            # ---- AllReduce the 4 KB stats across cores ----
            st_in = dram.tile([128, 2 * NB], dt.float32)
            st_out = dram.tile([128, 2 * NB], dt.float32)
            nc.sync.dma_start(st_in[:], st_loc[:])
            nc.gpsimd.collective_compute(
                "AllReduce", Alu.add,
                replica_groups=[list(range(ncores))],
                ins=[st_in.opt()], outs=[st_out.opt()],
            )
            st_g = stat.tile([128, 2 * NB], dt.float32)
            nc.sync.dma_start(st_g[:], st_out[:])

            # ---- threshold t = mean - b_bn * sqrt(var+eps) / w_bn ----
            inv_n = 1.0 / (n_loc * ncores)
            mean = stat.tile([128, NB], dt.float32)
            varpe = stat.tile([128, NB], dt.float32)
            tmp = stat.tile([128, NB], dt.float32)
            tmp2 = stat.tile([128, NB], dt.float32)
            sd = stat.tile([128, NB], dt.float32)
            thr = stat.tile([128, NB], dt.float32)
            nc.vector.tensor_scalar_mul(mean[:], st_g[:, 0:NB], inv_n)
            nc.vector.tensor_scalar_mul(varpe[:], st_g[:, NB:2 * NB], inv_n)
            nc.vector.tensor_tensor(tmp[:], mean[:], mean[:], Alu.mult)
            nc.vector.tensor_tensor(varpe[:], varpe[:], tmp[:], Alu.subtract)
            nc.vector.tensor_scalar_add(varpe[:], varpe[:], EPS_BN)
            # seed sqrt on ACT (low-precision table), then 2 Newton steps
            # with the accurate DVE reciprocal: s <- 0.5*(s + v/s)
            nc.scalar.activation(sd[:], varpe[:], Act.Sqrt)
            for _ in range(2):
                nc.vector.reciprocal(tmp[:], sd[:])
                nc.vector.tensor_tensor(tmp[:], varpe[:], tmp[:], Alu.mult)
                nc.vector.tensor_tensor(tmp[:], sd[:], tmp[:], Alu.add)
                nc.vector.tensor_scalar_mul(sd[:], tmp[:], 0.5)
            nc.vector.reciprocal(tmp[:], wbn[:])
            nc.vector.tensor_tensor(tmp2[:], bbn[:], sd[:], Alu.mult)
            nc.vector.tensor_tensor(tmp[:], tmp2[:], tmp[:], Alu.mult)
            nc.vector.tensor_tensor(thr[:], mean[:], tmp[:], Alu.subtract)

            # ---- pass 2: spike + matmul ----
            obatch = 4     # psum tiles per batched output DMA
            for j in range(nj):
                sps = []
                for b in range(NB):
                    xt = ld2.tile([128, f], dt.float32)
                    nc.sync.dma_start(
                        xt[:], xt_d[b * 128:(b + 1) * 128, j * f:(j + 1) * f])
                    sp = spk.tile([128, f], dt.float16)
                    nc.vector.tensor_scalar(
                        sp[:], xt[:], thr[:, b:b + 1], None, Alu.is_gt)
                    sps.append(sp)
                for s0 in range(0, nsub, obatch):
                    o_sb = outp.tile([128, obatch, O], dt.float16)
                    for si in range(obatch):
                        s = s0 + si
                        acc = ps.tile([128, O], dt.float32)
                        for b in range(NB):
                            nc.tensor.matmul(
                                acc[:], sps[b][:, s * 128:(s + 1) * 128],
                                wt_sb[:, b, :], start=(b == 0), stop=(b == NB - 1))
                        nc.scalar.copy(o_sb[:, si, :], acc[:])
                    row = j * f + s0 * 128
                    nc.sync.dma_start(
                        out_d[row:row + obatch * 128, :].rearrange(
                            "(s p) o -> p s o", p=128),
                        o_sb[:])

    nc.compile()
    return nc


_PROGRAM_CACHE = {}


def _get_program(n_loc=N_LOC, ncores=NCORES):
    key = (n_loc, ncores)
    if key not in _PROGRAM_CACHE:
        _PROGRAM_CACHE[key] = build_program(n_loc, ncores)
    return _PROGRAM_CACHE[key]


def _prep_inputs(x, weight_bn, bias_bn, weight_fc, ncores=NCORES):
    n = x.shape[0]
    n_loc = n // ncores
    wt = np.ascontiguousarray(weight_fc.T).astype(np.float16)
    wbn = np.ascontiguousarray(weight_bn.reshape(NB, 128).T)
    bbn = np.ascontiguousarray(bias_bn.reshape(NB, 128).T)
    xt_full = np.ascontiguousarray(x.T)  # [C, N]
    in_maps = []
    for i in range(ncores):
        xt_i = np.ascontiguousarray(xt_full[:, i * n_loc:(i + 1) * n_loc])
        in_maps.append({"xt": xt_i, "wt": wt, "wbn": wbn, "bbn": bbn})
    return in_maps


def kernel(x, weight_bn, bias_bn, weight_fc, bias_fc, _trace=False):
    x = np.asarray(x, dtype=np.float32)
    weight_bn = np.asarray(weight_bn, dtype=np.float32)
    bias_bn = np.asarray(bias_bn, dtype=np.float32)
    weight_fc = np.asarray(weight_fc, dtype=np.float32)
    bias_fc = np.asarray(bias_fc, dtype=np.float32)

    n = x.shape[0]
    ncores = NCORES
    n_loc = n // ncores
    nc = _get_program(n_loc, ncores)
    in_maps = _prep_inputs(x, weight_bn, bias_bn, weight_fc, ncores)
    res = run_bass_kernel_spmd(
        nc, in_maps, list(range(ncores)), trace=_trace)
    out = np.concatenate(
        [res.results[i]["out"] for i in range(ncores)], axis=0
    ).astype(np.float32)
    out += bias_fc[None, :]
    if _trace:
        kernel._last_results = res
    return out


if __name__ == "__main__":
    # quick self-check at reduced size (still 8 cores)
    rng = np.random.default_rng(0)
    n = int(os.environ.get("KERNEL_SELFTEST_N", 16384))
    x = rng.standard_normal((n, C)).astype(np.float32)
    wbn = rng.uniform(0.5, 1.5, C).astype(np.float32)
    bbn = (rng.standard_normal(C) * 0.1).astype(np.float32)
    wfc = rng.uniform(-1 / np.sqrt(C), 1 / np.sqrt(C), (O, C)).astype(np.float32)
    bfc = rng.uniform(-1 / np.sqrt(C), 1 / np.sqrt(C), O).astype(np.float32)

    mean = x.mean(axis=0)
    var = ((x - mean) ** 2).mean(axis=0)
    y = (x - mean) / np.sqrt(var + EPS_BN) * wbn + bbn
    spike = (y > 0).astype(np.float32)
    expected = spike @ wfc.T + bfc

    actual = kernel(x, wbn, bbn, wfc, bfc)
    err = np.abs(actual - expected).max() / np.abs(expected).max()
    print(f"selftest n={n}: rel_err={err:.3e}")


# revision 14
# speedup vs baseline: 1.3821x; 1.0380x over previous
"""BnFireFc fused kernel for 8 Trainium2 NeuronCores.

Math: training-mode BN over N, then heaviside spike, then FC:
    mean/var over axis 0 of x [N, C]
    y = (x - mean) * rsqrt(var + eps) * w_bn + b_bn
    spike = (y > 0)
    out = spike @ W_fc.T + b_fc

Since w_bn > 0 and rsqrt > 0, spike[n,c] == (x[n,c] > t[c]) with
    t[c] = mean[c] - b_bn[c] * sqrt(var[c] + eps) / w_bn[c]
so BN+Fire collapses to one per-channel threshold compare.

Distribution: data-parallel over N across 8 cores. Each core receives its
x-shard pre-transposed on host to [C, n_loc] so channels sit on SBUF
partitions (4 blocks of 128): pass 1 accumulates per-channel sum(x) and
sum(x^2) along the free dim (DVE / ACT accum_out), a 4 KB AllReduce merges
the partial stats, then pass 2 streams x^T again, compares against t[c]
(per-partition tensor_scalar, exact fp32) into an fp16 0/1 spike, and
accumulates out[n,:] = sum_c spike^T[c,n] * W_fc.T[c,:] on the PE in fp16
(spike values are exact in fp16; only W rounds). b_fc is added on host.

HBM-traffic optimizations: the last CACHE_J chunks of pass 1 stay resident
in SBUF and are not re-read in pass 2; the output is written as fp16 and
upcast on host; deep pass-2 load buffering lets the DMA engines prefetch
through the AllReduce sync bubble.
"""

import os
import numpy as np

import concourse.bass as bass
import concourse.tile as tile
from concourse import bacc, mybir
from concourse.bass_utils import run_bass_kernel_spmd

dt = mybir.dt
Alu = mybir.AluOpType
Act = mybir.ActivationFunctionType

NCORES = 8
N_FULL = 131072
C = 512          # cin
O = 512          # cout
NB = C // 128    # channel blocks on partitions
EPS_BN = 1e-5

N_LOC = N_FULL // NCORES   # 16384 rows per core
F1 = 2048                  # pass-1 tile free size (n elems)
F = 2048                   # pass-2 tile free size (n elems per chunk column)


def build_program(n_loc=N_LOC, ncores=NCORES):
    f1 = min(F1, n_loc)
    f = min(F, n_loc)
    nj1 = n_loc // f1             # pass-1 j-chunks
    nj = n_loc // f               # pass-2 j-chunks
    nsub = f // 128               # 128-row subtiles per chunk

    nc = bacc.Bacc("TRN2", target_bir_lowering=False, debug=False,
                   num_devices=ncores)

    xt_d = nc.dram_tensor("xt", [C, n_loc], dt.float32, kind="ExternalInput").ap()
    wt_d = nc.dram_tensor("wt", [C, O], dt.float16, kind="ExternalInput").ap()
    wbn_d = nc.dram_tensor("wbn", [128, NB], dt.float32, kind="ExternalInput").ap()
    bbn_d = nc.dram_tensor("bbn", [128, NB], dt.float32, kind="ExternalInput").ap()
    out_d = nc.dram_tensor("out", [O, n_loc], dt.float16, kind="ExternalOutput").ap()

    with tile.TileContext(nc) as tc:
        with (
            tc.tile_pool(name="ld1", bufs=3) as ld1,
            tc.tile_pool(name="ld2", bufs=15) as ld2,
            tc.tile_pool(name="spk", bufs=8) as spk,
            tc.tile_pool(name="wts", bufs=1) as wts,
            tc.tile_pool(name="stat", bufs=1) as stat,
            tc.tile_pool(name="outp", bufs=4) as outp,
            tc.tile_pool(name="ps", bufs=4, space="PSUM") as ps,
            tc.tile_pool(name="dram", bufs=1, space="DRAM") as dram,
        ):
            # ---- replicated weights ----
            wt_sb = wts.tile([128, NB, O], dt.float16)      # W_fc.T, c-blocked
            for b in range(NB):
                nc.sync.dma_start(wt_sb[:, b, :], wt_d[b * 128:(b + 1) * 128, :])
            wbn = wts.tile([128, NB], dt.float32)
            bbn = wts.tile([128, NB], dt.float32)
            nc.sync.dma_start(wbn[:], wbn_d[:])
            nc.sync.dma_start(bbn[:], bbn_d[:])

            # ---- pass 1: per-channel sum(x), sum(x^2) partials ----
            s1c = stat.tile([128, NB, nj1], dt.float32)
            s2c = stat.tile([128, NB, nj1], dt.float32)
            scr_d = stat.tile([128, 1], dt.bfloat16)
            scr_a = stat.tile([128, 1], dt.bfloat16)
            for j in range(nj1):
                for b in range(NB):
                    xt = ld1.tile([128, f1], dt.float32)
                    nc.sync.dma_start(
                        xt[:], xt_d[b * 128:(b + 1) * 128, j * f1:(j + 1) * f1])
                    nc.vector.tensor_scalar(
                        scr_d.broadcast_to((128, f1)), xt[:], 1.0, None,
                        Alu.mult, Alu.add, accum_out=s1c[:, b, j:j + 1])
                    nc.scalar.activation(
                        scr_a.broadcast_to((128, f1)), xt[:], Act.Square,
                        accum_out=s2c[:, b, j:j + 1])

            # reduce partials -> [128, NB], pack into [128, 2*NB]
            st_loc = stat.tile([128, 2 * NB], dt.float32)
            nc.vector.tensor_reduce(
                st_loc[:, 0:NB], s1c[:], axis=mybir.AxisListType.X, op=Alu.add)
            nc.vector.tensor_reduce(
                st_loc[:, NB:2 * NB], s2c[:], axis=mybir.AxisListType.X, op=Alu.add)

            # ---- AllReduce the 4 KB stats across cores ----
            st_in = dram.tile([128, 2 * NB], dt.float32)
            st_out = dram.tile([128, 2 * NB], dt.float32)
            nc.sync.dma_start(st_in[:], st_loc[:])
            nc.gpsimd.collective_compute(
                "AllReduce", Alu.add,
                replica_groups=[list(range(ncores))],
                ins=[st_in.opt()], outs=[st_out.opt()],
            )
            st_g = stat.tile([128, 2 * NB], dt.float32)
            nc.sync.dma_start(st_g[:], st_out[:])

            # ---- threshold t = mean - b_bn * sqrt(var+eps) / w_bn ----
            inv_n = 1.0 / (n_loc * ncores)
            mean = stat.tile([128, NB], dt.float32)
            varpe = stat.tile([128, NB], dt.float32)
            tmp = stat.tile([128, NB], dt.float32)
            tmp2 = stat.tile([128, NB], dt.float32)
            sd = stat.tile([128, NB], dt.float32)
            thr = stat.tile([128, NB], dt.float32)
            nc.vector.tensor_scalar_mul(mean[:], st_g[:, 0:NB], inv_n)
            nc.vector.tensor_scalar_mul(varpe[:], st_g[:, NB:2 * NB], inv_n)
            nc.vector.tensor_tensor(tmp[:], mean[:], mean[:], Alu.mult)
            nc.vector.tensor_tensor(varpe[:], varpe[:], tmp[:], Alu.subtract)
            nc.vector.tensor_scalar_add(varpe[:], varpe[:], EPS_BN)
            # seed sqrt on ACT (low-precision table), then 2 Newton steps
            # with the accurate DVE reciprocal: s <- 0.5*(s + v/s)
            nc.scalar.activation(sd[:], varpe[:], Act.Sqrt)
            for _ in range(2):
                nc.vector.reciprocal(tmp[:], sd[:])
                nc.vector.tensor_tensor(tmp[:], varpe[:], tmp[:], Alu.mult)
                nc.vector.tensor_tensor(tmp[:], sd[:], tmp[:], Alu.add)
                nc.vector.tensor_scalar_mul(sd[:], tmp[:], 0.5)
            nc.vector.reciprocal(tmp[:], wbn[:])
            nc.vector.tensor_tensor(tmp2[:], bbn[:], sd[:], Alu.mult)
            nc.vector.tensor_tensor(tmp[:], tmp2[:], tmp[:], Alu.mult)
            nc.vector.tensor_tensor(thr[:], mean[:], tmp[:], Alu.subtract)

            # ---- pass 2: spike + matmul (W stationary, out^T in PSUM) ----
            # out^T[o, n] = sum_c W^T[c, o] * spike^T[c, n].  Each LDWEIGHTS
            # of a [c128, o128] W block serves `pair` 512-wide n matmuls.
            fo = 512                       # moving free size (one PSUM bank)
            nblk = f // fo                 # 512-n blocks per chunk
            pair = 2                       # n-blocks sharing one LDWEIGHTS
            for j in range(nj):
                sps = []
                for b in range(NB):
                    xt = ld2.tile([128, f], dt.float32)
                    nc.sync.dma_start(
                        xt[:], xt_d[b * 128:(b + 1) * 128, j * f:(j + 1) * f])
                    sp = spk.tile([128, f], dt.float16)
                    nc.vector.tensor_scalar(
                        sp[:], xt[:], thr[:, b:b + 1], None, Alu.is_gt)
                    sps.append(sp)
                for g in range(nblk // pair):
                    for ob in range(NB):
                        acc = ps.tile([128, pair, fo], dt.float32)
                        for b in range(NB):
                            lhsT = wt_sb[:, b, ob * 128:(ob + 1) * 128]
                            for p in range(pair):
                                n0 = (g * pair + p) * fo
                                nc.tensor.matmul(
                                    acc[:, p, :], lhsT, sps[b][:, n0:n0 + fo],
                                    start=(b == 0), stop=(b == NB - 1))
                        o_sb = outp.tile([128, pair * fo], dt.float16)
                        nc.scalar.copy(o_sb[:], acc[:].rearrange("p a b -> p (a b)"))
                        n0 = j * f + g * pair * fo
                        nc.sync.dma_start(
                            out_d[ob * 128:(ob + 1) * 128, n0:n0 + pair * fo],
                            o_sb[:])

    nc.compile()
    return nc


_PROGRAM_CACHE = {}


def _get_program(n_loc=N_LOC, ncores=NCORES):
    key = (n_loc, ncores)
    if key not in _PROGRAM_CACHE:
        _PROGRAM_CACHE[key] = build_program(n_loc, ncores)
    return _PROGRAM_CACHE[key]


def _prep_inputs(x, weight_bn, bias_bn, weight_fc, ncores=NCORES):
    n = x.shape[0]
    n_loc = n // ncores
    wt = np.ascontiguousarray(weight_fc.T).astype(np.float16)
    wbn = np.ascontiguousarray(weight_bn.reshape(NB, 128).T)
    bbn = np.ascontiguousarray(bias_bn.reshape(NB, 128).T)
    xt_full = np.ascontiguousarray(x.T)  # [C, N]
    in_maps = []
    for i in range(ncores):
        xt_i = np.ascontiguousarray(xt_full[:, i * n_loc:(i + 1) * n_loc])
        in_maps.append({"xt": xt_i, "wt": wt, "wbn": wbn, "bbn": bbn})
    return in_maps


def kernel(x, weight_bn, bias_bn, weight_fc, bias_fc, _trace=False):
    x = np.asarray(x, dtype=np.float32)
    weight_bn = np.asarray(weight_bn, dtype=np.float32)
    bias_bn = np.asarray(bias_bn, dtype=np.float32)
    weight_fc = np.asarray(weight_fc, dtype=np.float32)
    bias_fc = np.asarray(bias_fc, dtype=np.float32)

    n = x.shape[0]
    ncores = NCORES
    n_loc = n // ncores
    nc = _get_program(n_loc, ncores)
    in_maps = _prep_inputs(x, weight_bn, bias_bn, weight_fc, ncores)
    res = run_bass_kernel_spmd(
        nc, in_maps, list(range(ncores)), trace=_trace)
    out = np.concatenate(
        [res.results[i]["out"].T for i in range(ncores)], axis=0
    ).astype(np.float32)
    out += bias_fc[None, :]
    if _trace:
        kernel._last_results = res
    return out


if __name__ == "__main__":
    # quick self-check at reduced size (still 8 cores)
    rng = np.random.default_rng(0)
    n = int(os.environ.get("KERNEL_SELFTEST_N", 16384))
    x = rng.standard_normal((n, C)).astype(np.float32)
    wbn = rng.uniform(0.5, 1.5, C).astype(np.float32)
    bbn = (rng.standard_normal(C) * 0.1).astype(np.float32)
    wfc = rng.uniform(-1 / np.sqrt(C), 1 / np.sqrt(C), (O, C)).astype(np.float32)
    bfc = rng.uniform(-1 / np.sqrt(C), 1 / np.sqrt(C), O).astype(np.float32)

    mean = x.mean(axis=0)
    var = ((x - mean) ** 2).mean(axis=0)
    y = (x - mean) / np.sqrt(var + EPS_BN) * wbn + bbn
    spike = (y > 0).astype(np.float32)
    expected = spike @ wfc.T + bfc

    actual = kernel(x, wbn, bbn, wfc, bfc)
    err = np.abs(actual - expected).max() / np.abs(expected).max()
    print(f"selftest n={n}: rel_err={err:.3e}")
